# revision 25
# baseline (speedup 1.0000x reference)
"""3-layer GAT on 8 Trainium2 NeuronCores (Bass/Tile) — v3.

Sharding: nodes by contiguous range (6250/core); edges by dst range. Per layer:
dense phase computes [feat|el|er] = h @ [W|W.al|W.ar] for local nodes (bf16
table rows, el kept f32 inside the row) -> AllGather the node table in two
half-shards (the first one overlaps with the second half of the dense loop) ->
edge phase gathers table[src] rows (dma_gather, int16 indices, 32768-row
table split, emitted one window ahead of the consumers), builds one-hot(dst)
tiles on DVE in bf16, broadcasts er via transposed one-hot matmuls, computes
exp(leaky_relu(el+er)) on DVE/ACT, and aggregates (weighted feature sum +
softmax denominator) with one bf16 matmul chain per 128-dst-node window into
PSUM. Epilogue normalizes, adds residual, applies ELU (or the head-mean for
the output layer).

The dense phase feeds the TensorE via HWDGE DMA-transpose loads of the bf16
activations. Softmax skips the segment-max subtraction: logits are O(1) so
exp() cannot overflow.
"""
import sys

sys.path.insert(0, "/opt/trn_rl_repo")

import os as _os
import numpy as np
import ml_dtypes

BF16 = ml_dtypes.bfloat16

# ---- problem constants (nn_GAT_3951369912452) ----
N = 50000
E = 800000
IN = 256
HID = 64
H = 4
C = 40
SLOPE = 0.2
NCORES = 8
NLOC = N // NCORES          # 6250
P = 128
W = (NLOC + P - 1) // P     # 49 windows/core
NLOCP = W * P               # 6272 (padded local rows)
H1W = 25                    # windows in AllGather half 1
H1LOC = H1W * P             # 3200
H2LOC = NLOC - H1LOC        # 3050
SPLIT = 32768               # int16 gather index limit

F0 = H * HID                # 256 feat width, layers 0/1
F2 = H * C                  # 160 feat width, layer 2
ROW0 = 384                  # bf16 table row elems, layers 0/1 (768B)
ROW2 = 256                  # layer 2 (512B)
EL0, EL2 = 256, 160         # el slot offset (bf16 elems; 4 f32 live there)
ER0, ER2 = 264, 168         # er slot offset (4 more f32)
RHS0 = 260                  # matmul rhs cols (feat + ex), layers 0/1
RHS2 = 164                  # layer 2
NBUF = 3                    # gather buffer ring
LA = 1                      # gather lookahead (windows)

_CACHE = {}
AGSPLIT = _os.environ.get("GAT_NOAGSPLIT", "") != "1"


# ======================= host preprocessing =======================

def _remap(n):
    """Global node id -> table row under the two-half AllGather layout."""
    if not AGSPLIT:
        return n
    r, ln = n // NLOC, n % NLOC
    return np.where(ln < H1LOC, r * H1LOC + ln,
                    NCORES * H1LOC + r * H2LOC + (ln - H1LOC))


def _fold_w(Wm, al, ar):
    Hh, D = al.shape
    Wal = np.stack([Wm[:, h * D:(h + 1) * D] @ al[h] for h in range(Hh)], axis=1)
    War = np.stack([Wm[:, h * D:(h + 1) * D] @ ar[h] for h in range(Hh)], axis=1)
    return Wal.astype(np.float32), War.astype(np.float32)


def _wrap16(block):
    """int16 idx list (cap,) -> dma_gather wrapped layout (128, cap//16)."""
    cap = block.shape[0]
    wb = block.reshape(cap // 16, 16).T
    return np.tile(wb, (8, 1)).astype(np.int16)


def _preprocess(inputs):
    x = np.asarray(inputs["x"], np.float32)
    src = np.asarray(inputs["src"], np.int64)
    dst = np.asarray(inputs["dst"], np.int64)

    Wcat = []
    for l, (Wm, al, ar) in enumerate(
        [(inputs["W0"], inputs["al0"], inputs["ar0"]),
         (inputs["W1"], inputs["al1"], inputs["ar1"]),
         (inputs["W2"], inputs["al2"], inputs["ar2"])]
    ):
        Wm = np.asarray(Wm, np.float32)
        Wal, War = _fold_w(Wm, np.asarray(al, np.float32), np.asarray(ar, np.float32))
        parts = [Wm, Wal, War]
        if l == 2:
            # residual projection, pre-scaled by the head-mean 1/H factor
            parts.append(np.asarray(inputs["Wres2"], np.float32) / H)
        Wcat.append(np.ascontiguousarray(np.concatenate(parts, axis=1)).astype(BF16))

    rsrc = _remap(src)
    # the appended self-loop block (one per node) is handled locally in the
    # epilogue — exclude exactly those edges from the gather lists
    selfapp = np.zeros(E, bool)
    selfapp[E - N:] = True
    order = np.argsort(dst, kind="stable")
    ds = dst[order]
    ss = rsrc[order]
    sf = selfapp[order]

    # per (core, window) edge lists
    per_core = []
    KA = KB = 1
    for r in range(NCORES):
        lo = r * NLOC
        m = (ds >= lo) & (ds < lo + NLOC) & ~sf
        ld = ds[m] - lo
        ls = ss[m]
        wins = []
        for w in range(W):
            wm = (ld >= w * P) & (ld < (w + 1) * P)
            dw = ld[wm] - w * P
            sw = ls[wm]
            a = sw < SPLIT
            sa, da = sw[a], dw[a]
            sb, db = sw[~a] - SPLIT, dw[~a]
            wins.append((sa, da, sb, db))
            KA = max(KA, -(-max(len(sa), 1) // P))
            KB = max(KB, -(-max(len(sb), 1) // P))
        per_core.append(wins)
    KT = KA + KB

    in_maps = []
    for r in range(NCORES):
        idxA = np.full((W, KA * P), -1, np.int16)
        idxB = np.full((W, KB * P), -1, np.int16)
        drel = np.full((W, KT * P), -1.0, np.float32)
        meta = np.zeros((1, 2 * W), np.int32)
        for w, (sa, da, sb, db) in enumerate(per_core[r]):
            na, nb = len(sa), len(sb)
            idxA[w, :na] = sa.astype(np.int16)
            idxB[w, :nb] = sb.astype(np.int16)
            if na == 0:
                idxA[w, 0] = 0
            if nb == 0:
                idxB[w, 0] = 0
            meta[0, w] = max(na, 1)
            meta[0, W + w] = max(nb, 1)
            drel[w, :na] = da.astype(np.float32)
            drel[w, KA * P:KA * P + nb] = db.astype(np.float32)

        xp = np.zeros((NLOCP, IN), np.float32)
        xp[:NLOC] = x[r * NLOC:(r + 1) * NLOC]
        in_maps.append({
            "x": xp.astype(BF16),
            "Wcat0": Wcat[0], "Wcat1": Wcat[1], "Wcat2": Wcat[2],
            "idxA": np.hstack([_wrap16(idxA[w]) for w in range(W)]),
            "idxB": np.hstack([_wrap16(idxB[w]) for w in range(W)]),
            "drel": np.hstack([drel[w].reshape(KT, P).T for w in range(W)]).astype(BF16),
            "meta": meta,
        })

    meta_prog = {"KA": KA, "KB": KB}
    return in_maps, meta_prog


# ======================= device program =======================

def _build(meta_prog):
    import concourse.bass as bass
    import concourse.bacc as bacc
    import concourse.mybir as mybir
    import concourse.tile as tile
    from concourse.masks import make_identity

    KA, KB = meta_prog["KA"], meta_prog["KB"]
    KT = KA + KB
    f32 = mybir.dt.float32
    bf16 = mybir.dt.bfloat16
    i16 = mybir.dt.int16
    i32 = mybir.dt.int32
    AF = mybir.ActivationFunctionType
    OP = mybir.AluOpType

    nc = bacc.Bacc("TRN2", target_bir_lowering=False, debug=False,
                   num_devices=NCORES)

    ROWS = (ROW0, ROW0, ROW2)
    ELS = (EL0, EL0, EL2)
    FS = (F0, F0, F2)
    RHSS = (RHS0, RHS0, RHS2)
    DCOLS = (F0 + 8, F0 + 8, F2 + 8 + F2)

    # ---- I/O ----
    x_d = nc.dram_tensor("x", [NLOCP, F0], bf16, kind="ExternalInput")
    Wc_d = [nc.dram_tensor(f"Wcat{l}", [IN, DCOLS[l]], bf16, kind="ExternalInput")
            for l in range(3)]
    idxA_d = nc.dram_tensor("idxA", [P, W * KA * 8], i16, kind="ExternalInput")
    idxB_d = nc.dram_tensor("idxB", [P, W * KB * 8], i16, kind="ExternalInput")
    drel_d = nc.dram_tensor("drel", [P, W * KT], bf16, kind="ExternalInput")
    meta_d = nc.dram_tensor("meta", [1, 2 * W], i32, kind="ExternalInput")
    out_d = nc.dram_tensor("out", [NLOC, C], f32, kind="ExternalOutput")

    # ---- internal DRAM ----
    tin = [nc.dram_tensor(f"tin{l}", [NLOC, ROWS[l]], bf16) for l in range(3)]
    tab = [nc.dram_tensor(f"tab{l}", [N, ROWS[l]], bf16, addr_space="Shared")
           for l in range(3)]
    h_d = [x_d,
           nc.dram_tensor("h1", [NLOCP, F0], bf16),
           nc.dram_tensor("h2", [NLOCP, F0], bf16)]
    res2_d = nc.dram_tensor("res2", [NLOC, F2], f32)

    with tile.TileContext(nc) as tc:
        with (
            tc.tile_pool(name="const", bufs=1) as cp,
            tc.tile_pool(name="work", bufs=2) as wp,
            tc.tile_pool(name="psum", bufs=2, space="PSUM") as pp,
        ):
            # ---- persistent tiles ----
            iota_f = cp.tile([P, P], f32)
            nc.gpsimd.iota(iota_f[:], pattern=[[1, P]], base=0,
                           channel_multiplier=0,
                           allow_small_or_imprecise_dtypes=True)
            iota_b = cp.tile([P, P], bf16)
            nc.vector.tensor_copy(out=iota_b[:], in_=iota_f[:])
            ident_f = cp.tile([P, P], f32)
            make_identity(nc, ident_f[:])
            ident_b = cp.tile([P, P], bf16)
            nc.vector.tensor_copy(out=ident_b[:], in_=ident_f[:])

            idxA_t = cp.tile([P, W * KA * 8], i16)
            nc.sync.dma_start(idxA_t[:], idxA_d[:, :])
            idxB_t = cp.tile([P, W * KB * 8], i16)
            nc.sync.dma_start(idxB_t[:], idxB_d[:, :])
            drel_t = cp.tile([P, W * KT], bf16)
            nc.sync.dma_start(drel_t[:], drel_d[:, :])
            meta_t = cp.tile([1, 2 * W], i32)
            nc.sync.dma_start(meta_t[:], meta_d[:, :])

            Wc_t = []
            for l in range(3):
                chunks = []
                for k in range(2):
                    t = cp.tile([P, DCOLS[l]], bf16, tag=f"wc{l}{k}")
                    nc.sync.dma_start(t[:], Wc_d[l][k * P:(k + 1) * P, :])
                    chunks.append(t)
                Wc_t.append(chunks)

            # gather destinations: ring of NBUF buffers; zeroed once so pad
            # lanes (not overwritten by the exact-count gathers) stay finite
            G_bufs = []
            for i in range(NBUF):
                g = cp.tile([P, KT * ROW0], bf16, tag=f"G{i}")
                nc.vector.memset(g[:, :].bitcast(f32), 0.0)
                G_bufs.append(g)

            regs = [nc.gpsimd.alloc_register(f"gr{i}") for i in range(6)]
            er_all = cp.tile([P, W, 4], f32)
            el_all = cp.tile([P, W, 4], f32)

            # zero the h1/h2 DRAM pad rows once: the dense-phase transposed
            # loads read them, and a NaN there would poison er_all via the
            # full-partition er matmul contraction
            zpad = cp.tile([NLOCP - NLOC, F0], bf16, tag="zpad")
            nc.vector.memset(zpad[:].bitcast(f32), 0.0)
            nc.sync.dma_start(h_d[1][NLOC:NLOCP, :], zpad[:])
            nc.sync.dma_start(h_d[2][NLOC:NLOCP, :], zpad[:])

            import os
            DBG = os.environ.get("GAT_DBG", "") == "1"
            DBG_L = int(os.environ.get("GAT_DBG_L", "0"))
            DBG_W = int(os.environ.get("GAT_DBG_W", "0"))

            def _tap(name, ap, shape, dtype):
                d = nc.dram_tensor(name, shape, dtype, kind="ExternalOutput")
                nc.sync.dma_start(d[tuple(slice(0, s) for s in shape)], ap)

            def dense_phase(l):
                F = FS[l]
                hview = h_d[l].rearrange("n (k p) -> n k p", p=P)
                for w in range(W):
                    nw = min(P, NLOC - w * P)
                    hT = wp.tile([P, 2, P], bf16, tag="hT")
                    for k in range(2):
                        nc.sync.dma_start(out=hT[:, k, :],
                                          in_=hview[w * P:(w + 1) * P, k, :],
                                          transpose=True)
                    psd = pp.tile([P, DCOLS[l]], f32, tag="psd")
                    for k in range(2):
                        nc.tensor.matmul(out=psd[:], lhsT=hT[:, k, :],
                                         rhs=Wc_t[l][k][:],
                                         start=(k == 0), stop=(k == 1))
                    do = wp.tile([P, ROWS[l]], bf16, tag="do")
                    nc.vector.tensor_copy(out=do[:, 0:F], in_=psd[:, 0:F])
                    nc.vector.tensor_copy(
                        out=do[:, ELS[l]:ELS[l] + 8].bitcast(f32),
                        in_=psd[:, F:F + 4])
                    nc.vector.tensor_copy(out=er_all[:, w, :],
                                          in_=psd[:, F + 4:F + 8])
                    nc.vector.tensor_copy(out=el_all[:, w, :],
                                          in_=psd[:, F:F + 4])
                    nc.scalar.dma_start(tin[l][w * P:w * P + nw, :], do[0:nw, :])
                    if DBG and l == DBG_L and w == DBG_W:
                        _tap("d_do", do[:, :], [P, ROWS[l]], bf16)
                        _tap("d_hT", hT[:, :, :], [P, 2, P], bf16)
                        psc = wp.tile([P, DCOLS[l]], f32, tag="d_psc")
                        nc.vector.tensor_copy(out=psc[:], in_=psd[:])
                        _tap("d_psd", psc[:], [P, DCOLS[l]], f32)
                    if l == 2:
                        r2 = wp.tile([P, F2], f32, tag="r2")
                        nc.vector.tensor_copy(out=r2[:], in_=psd[:, F2 + 8:F2 + 8 + F2])
                        nc.scalar.dma_start(res2_d[w * P:w * P + nw, :], r2[0:nw, :])
                    if AGSPLIT and w == H1W - 1:
                        nc.gpsimd.collective_compute(
                            "AllGather", OP.bypass,
                            replica_groups=[list(range(NCORES))],
                            ins=[tin[l][0:H1LOC, :].opt()],
                            outs=[tab[l][0:NCORES * H1LOC, :].opt()])
                    elif w == W - 1:
                        if AGSPLIT:
                            nc.gpsimd.collective_compute(
                                "AllGather", OP.bypass,
                                replica_groups=[list(range(NCORES))],
                                ins=[tin[l][H1LOC:NLOC, :].opt()],
                                outs=[tab[l][NCORES * H1LOC:N, :].opt()])
                        else:
                            nc.gpsimd.collective_compute(
                                "AllGather", OP.bypass,
                                replica_groups=[list(range(NCORES))],
                                ins=[tin[l][:, :].opt()],
                                outs=[tab[l][:, :].opt()])

            def edge_phase(l):
                ROW, EL, ER, F, RHSW = ROWS[l], ELS[l], ELS[l] + 8, FS[l], RHSS[l]
                D = F // H

                if l == 2:
                    # G stale bytes from the ROW0 layout would be reinterpreted
                    # at ROW2 stride (el slots landing mid-feat -> huge bitcast
                    # values -> exp overflow -> 0*inf NaN). Re-zero the ring.
                    for g in G_bufs:
                        nc.vector.memset(g[:, :].bitcast(f32), 0.0)

                def gath(w):
                    bufi = w % NBUF
                    G = G_bufs[bufi][:, 0:KT * ROW].rearrange(
                        "p (t c) -> p t c", c=ROW)
                    rA = regs[(2 * w) % 6]
                    rB = regs[(2 * w + 1) % 6]
                    nc.gpsimd.reg_load(rA, meta_t[0:1, w:w + 1])
                    nc.gpsimd.dma_gather(
                        out_ap=G[:, 0:KA, :], in_ap=tab[l][0:SPLIT, :],
                        idxs_ap=idxA_t[:, w * KA * 8:(w + 1) * KA * 8],
                        num_idxs=KA * P, num_idxs_reg=rA, elem_size=ROW,
                        single_packet=False)
                    nc.gpsimd.reg_load(rB, meta_t[0:1, W + w:W + w + 1])
                    nc.gpsimd.dma_gather(
                        out_ap=G[:, KA:KT, :], in_ap=tab[l][SPLIT:N, :],
                        idxs_ap=idxB_t[:, w * KB * 8:(w + 1) * KB * 8],
                        num_idxs=KB * P, num_idxs_reg=rB, elem_size=ROW,
                        single_packet=False)

                for w in range(LA):
                    gath(w)
                for w in range(W):
                    if w + LA < W:
                        gath(w + LA)
                    nw = min(P, NLOC - w * P)
                    bufi = w % NBUF
                    G = G_bufs[bufi][:, 0:KT * ROW].rearrange(
                        "p (t c) -> p t c", c=ROW)

                    oh = wp.tile([P, KT, P], bf16, tag="oh")
                    nc.vector.tensor_tensor(
                        out=oh[:, :, :],
                        in0=iota_b[:, None, :].to_broadcast([P, KT, P]),
                        in1=drel_t[:, w * KT:(w + 1) * KT, None].to_broadcast([P, KT, P]),
                        op=OP.is_equal)

                    # er broadcast to edges: per tile, transpose the one-hot
                    # and multiply by this window's per-node er (SBUF-resident)
                    erw_t = wp.tile([P, 4], bf16, tag="erwb")
                    nc.vector.tensor_copy(out=erw_t[:], in_=er_all[:, w, :])
                    er_ps = pp.tile([P, KT, 4], f32, tag="er_ps")
                    for t in range(KT):
                        oht_ps = pp.tile([P, P], bf16, tag="pst")
                        nc.tensor.transpose(out=oht_ps[:], in_=oh[:, t, :],
                                            identity=ident_b[:])
                        ohT = wp.tile([P, P], bf16, tag="ohT")
                        nc.vector.tensor_copy(out=ohT[:], in_=oht_ps[:])
                        nc.tensor.matmul(out=er_ps[:, t, :], lhsT=ohT[:],
                                         rhs=erw_t[:], start=True, stop=True)

                    ext = wp.tile([P, KT, 4], f32, tag="ext")
                    nc.vector.tensor_add(
                        ext[:, :, :],
                        G[:, :, EL:EL + 8].bitcast(f32),
                        er_ps[:, :, :])
                    nc.vector.scalar_tensor_tensor(
                        out=ext[:, :, :], in0=ext[:, :, :], scalar=SLOPE,
                        in1=ext[:, :, :], op0=OP.mult, op1=OP.max)
                    extb = wp.tile([P, KT, 4], bf16, tag="extb")
                    nc.scalar.activation(extb[:, :, :], ext[:, :, :], AF.Exp)

                    rhs = wp.tile([P, KT, RHSW], bf16, tag="rhs")
                    nc.vector.tensor_tensor(
                        out=rhs[:, :, 0:F].rearrange("p t (h d) -> p t h d", h=H),
                        in0=G[:, :, 0:F].rearrange("p t (h d) -> p t h d", h=H),
                        in1=extb[:, :, :, None].to_broadcast([P, KT, H, D]),
                        op=OP.mult)
                    nc.vector.tensor_copy(out=rhs[:, :, F:F + 4], in_=extb[:, :, :])

                    psw = pp.tile([P, RHSW], f32, tag="psw")
                    for t in range(KT):
                        nc.tensor.matmul(out=psw[:, :], lhsT=oh[:, t, :],
                                         rhs=rhs[:, t, :],
                                         start=(t == 0), stop=(t == KT - 1))

                    if DBG and l == DBG_L and w == DBG_W:
                        _tap("d_G", G[:, :, :], [P, KT, ROW], bf16)
                        erc = wp.tile([P, KT, 4], f32, tag="d_erc")
                        nc.vector.tensor_copy(out=erc[:, :, :], in_=er_ps[:, :, :])
                        _tap("d_erps", erc[:, :, :], [P, KT, 4], f32)
                        _tap("d_ext", ext[:, :, :], [P, KT, 4], f32)
                        _tap("d_extb", extb[:, :, :], [P, KT, 4], bf16)
                        _tap("d_oh", oh[:, :, :], [P, KT, P], bf16)
                        _tap("d_rhs", rhs[:, :, :], [P, KT, RHSW], bf16)
                        psb = wp.tile([P, RHSW], f32, tag="d_psb")
                        nc.vector.tensor_copy(out=psb[:], in_=psw[:, :])
                        _tap("d_psw", psb[:], [P, RHSW], f32)

                    # self-loop contribution (excluded from the gather lists):
                    # feat/el/er of the window's own nodes are all local
                    exts = wp.tile([P, 4], f32, tag="exts")
                    nc.vector.tensor_add(exts[:], el_all[:, w, :], er_all[:, w, :])
                    nc.vector.scalar_tensor_tensor(
                        out=exts[:], in0=exts[:], scalar=SLOPE,
                        in1=exts[:], op0=OP.mult, op1=OP.max)
                    nc.scalar.activation(exts[:], exts[:], AF.Exp)
                    fs = wp.tile([P, F], bf16, tag="fs")
                    nc.scalar.dma_start(fs[0:nw, :], tin[l][w * P:w * P + nw, 0:F])
                    num = wp.tile([P, F], f32, tag="num")
                    nc.vector.tensor_tensor(
                        out=num[:].rearrange("p (h d) -> p h d", h=H),
                        in0=fs[:].rearrange("p (h d) -> p h d", h=H),
                        in1=exts[:, :, None].to_broadcast([P, H, D]),
                        op=OP.mult)
                    nc.vector.tensor_add(num[:], num[:], psw[:, 0:F])

                    dn = wp.tile([P, 4], f32, tag="dn")
                    nc.vector.tensor_add(dn[:], psw[:, F:F + 4], exts[:])
                    if l < 2:
                        nc.vector.tensor_scalar_max(dn[:], dn[:], 1e-30)
                    else:
                        nc.vector.tensor_scalar(dn[:], dn[:],
                                                1e-30, float(H), OP.max, OP.mult)
                    rec = wp.tile([P, 4], f32, tag="rec")
                    nc.vector.reciprocal(rec[:], dn[:])

                    of = wp.tile([P, F], f32, tag="of")
                    nc.vector.tensor_tensor(
                        out=of[:].rearrange("p (h d) -> p h d", h=H),
                        in0=num[:].rearrange("p (h d) -> p h d", h=H),
                        in1=rec[:, :, None].to_broadcast([P, H, D]),
                        op=OP.mult)
                    if l == 1:
                        rt = wp.tile([P, F0], bf16, tag="rt")
                        nc.scalar.dma_start(rt[0:nw, :], h_d[1][w * P:w * P + nw, :])
                        nc.vector.tensor_add(of[:], of[:], rt[:])
                    elif l == 2:
                        rt2 = wp.tile([P, F2], f32, tag="rt2")
                        nc.scalar.dma_start(rt2[0:nw, :], res2_d[w * P:w * P + nw, :])
                        nc.vector.tensor_add(of[:], of[:], rt2[:])

                    if l < 2:
                        # ELU: out = (x - 1 - min(x,0)) + exp(min(x,0))
                        t0 = wp.tile([P, F0], f32, tag="t0")
                        nc.vector.tensor_scalar_min(t0[:], of[:], 0.0)
                        o1 = wp.tile([P, F0], f32, tag="o1")
                        nc.vector.scalar_tensor_tensor(
                            out=o1[:], in0=of[:], scalar=-1.0, in1=t0[:],
                            op0=OP.add, op1=OP.subtract)
                        nc.scalar.activation(t0[:], t0[:], AF.Exp)
                        hb = wp.tile([P, F0], bf16, tag="hb")
                        nc.vector.tensor_add(hb[:], o1[:], t0[:])
                        nc.scalar.dma_start(h_d[l + 1][w * P:w * P + nw, :],
                                            hb[0:nw, :])
                    else:
                        msum = wp.tile([P, C], f32, tag="msum")
                        nc.vector.tensor_reduce(
                            msum[:],
                            of[:].rearrange("p (h c) -> p c h", h=H),
                            axis=mybir.AxisListType.X, op=OP.add)
                        nc.scalar.dma_start(out_d[w * P:w * P + nw, :], msum[0:nw, :])

            for l in range(3):
                dense_phase(l)
                edge_phase(l)

            if DBG:
                d_h1 = nc.dram_tensor("d_h1", [NLOCP, F0], bf16,
                                      kind="ExternalOutput")
                nc.sync.dma_start(d_h1[:, :], h_d[1][:, :])
                d_h2 = nc.dram_tensor("d_h2", [NLOCP, F0], bf16,
                                      kind="ExternalOutput")
                nc.sync.dma_start(d_h2[:, :], h_d[2][:, :])
                d_r2 = nc.dram_tensor("d_r2", [NLOC, F2], f32,
                                      kind="ExternalOutput")
                nc.sync.dma_start(d_r2[:, :], res2_d[:, :])

    nc.compile()
    return nc


# ======================= entry point =======================

def kernel(**inputs) -> np.ndarray:
    from concourse.bass_utils import run_bass_kernel_spmd

    in_maps, meta_prog = _preprocess(inputs)
    key = (meta_prog["KA"], meta_prog["KB"])
    if key not in _CACHE:
        _CACHE[key] = _build(meta_prog)
    nc = _CACHE[key]
    res = run_bass_kernel_spmd(nc, in_maps, core_ids=list(range(NCORES)))
    return np.concatenate([r["out"] for r in res.results], axis=0)


# revision 34
# speedup vs baseline: 1.0214x; 1.0214x over previous
"""3-layer GAT on 8 Trainium2 NeuronCores (Bass/Tile) — v3.

Sharding: nodes by contiguous range (6250/core); edges by dst range. Per layer:
dense phase computes [feat|el|er] = h @ [W|W.al|W.ar] for local nodes (bf16
table rows, el kept f32 inside the row) -> AllGather the node table in two
half-shards (the first one overlaps with the second half of the dense loop) ->
edge phase gathers table[src] rows (dma_gather, int16 indices, 32768-row
table split, emitted one window ahead of the consumers), builds one-hot(dst)
tiles on DVE in bf16, broadcasts er via transposed one-hot matmuls, computes
exp(leaky_relu(el+er)) on DVE/ACT, and aggregates (weighted feature sum +
softmax denominator) with one bf16 matmul chain per 128-dst-node window into
PSUM. Epilogue normalizes, adds residual, applies ELU (or the head-mean for
the output layer).

The dense phase feeds the TensorE via HWDGE DMA-transpose loads of the bf16
activations. Softmax skips the segment-max subtraction: logits are O(1) so
exp() cannot overflow.
"""
import sys

sys.path.insert(0, "/opt/trn_rl_repo")

import os as _os
import numpy as np
import ml_dtypes

BF16 = ml_dtypes.bfloat16

# ---- problem constants (nn_GAT_3951369912452) ----
N = 50000
E = 800000
IN = 256
HID = 64
H = 4
C = 40
SLOPE = 0.2
NCORES = 8
NLOC = N // NCORES          # 6250
P = 128
W = (NLOC + P - 1) // P     # 49 windows/core
NLOCP = W * P               # 6272 (padded local rows)
H1W = 25                    # windows in AllGather half 1
H1LOC = H1W * P             # 3200
H2LOC = NLOC - H1LOC        # 3050
SPLIT = 32768               # int16 gather index limit

F0 = H * HID                # 256 feat width, layers 0/1
F2 = H * C                  # 160 feat width, layer 2
ROW0 = 384                  # bf16 table row elems, layers 0/1 (768B)
ROW2 = 256                  # layer 2 (512B)
EL0, EL2 = 256, 160         # el slot offset (bf16 elems; 4 f32 live there)
ER0, ER2 = 264, 168         # er slot offset (4 more f32)
RHS0 = 260                  # matmul rhs cols (feat + ex), layers 0/1
RHS2 = 164                  # layer 2
NBUF = 4                    # gather buffer ring
LA = 2                      # gather lookahead (windows)

_CACHE = {}
AGSPLIT = _os.environ.get("GAT_NOAGSPLIT", "") != "1"


# ======================= host preprocessing =======================

def _remap(n):
    """Global node id -> table row under the two-half AllGather layout."""
    if not AGSPLIT:
        return n
    r, ln = n // NLOC, n % NLOC
    return np.where(ln < H1LOC, r * H1LOC + ln,
                    NCORES * H1LOC + r * H2LOC + (ln - H1LOC))


def _fold_w(Wm, al, ar):
    Hh, D = al.shape
    Wal = np.stack([Wm[:, h * D:(h + 1) * D] @ al[h] for h in range(Hh)], axis=1)
    War = np.stack([Wm[:, h * D:(h + 1) * D] @ ar[h] for h in range(Hh)], axis=1)
    return Wal.astype(np.float32), War.astype(np.float32)


def _wrap16(block):
    """int16 idx list (cap,) -> dma_gather wrapped layout (128, cap//16)."""
    cap = block.shape[0]
    wb = block.reshape(cap // 16, 16).T
    return np.tile(wb, (8, 1)).astype(np.int16)


def _preprocess(inputs):
    x = np.asarray(inputs["x"], np.float32)
    src = np.asarray(inputs["src"], np.int64)
    dst = np.asarray(inputs["dst"], np.int64)

    Wcat = []
    for l, (Wm, al, ar) in enumerate(
        [(inputs["W0"], inputs["al0"], inputs["ar0"]),
         (inputs["W1"], inputs["al1"], inputs["ar1"]),
         (inputs["W2"], inputs["al2"], inputs["ar2"])]
    ):
        Wm = np.asarray(Wm, np.float32)
        Wal, War = _fold_w(Wm, np.asarray(al, np.float32), np.asarray(ar, np.float32))
        parts = [Wm, Wal, War]
        if l == 2:
            # residual projection, pre-scaled by the head-mean 1/H factor
            parts.append(np.asarray(inputs["Wres2"], np.float32) / H)
        Wcat.append(np.ascontiguousarray(np.concatenate(parts, axis=1)).astype(BF16))

    rsrc = _remap(src)
    # the appended self-loop block (one per node) is handled locally in the
    # epilogue — exclude exactly those edges from the gather lists
    selfapp = np.zeros(E, bool)
    selfapp[E - N:] = True
    order = np.argsort(dst, kind="stable")
    ds = dst[order]
    ss = rsrc[order]
    sf = selfapp[order]

    # per (core, window) edge lists
    per_core = []
    KA = KB = 1
    for r in range(NCORES):
        lo = r * NLOC
        m = (ds >= lo) & (ds < lo + NLOC) & ~sf
        ld = ds[m] - lo
        ls = ss[m]
        wins = []
        for w in range(W):
            wm = (ld >= w * P) & (ld < (w + 1) * P)
            dw = ld[wm] - w * P
            sw = ls[wm]
            a = sw < SPLIT
            sa, da = sw[a], dw[a]
            sb, db = sw[~a] - SPLIT, dw[~a]
            wins.append((sa, da, sb, db))
            KA = max(KA, -(-max(len(sa), 1) // P))
            KB = max(KB, -(-max(len(sb), 1) // P))
        per_core.append(wins)
    KT = KA + KB

    in_maps = []
    for r in range(NCORES):
        idxA = np.full((W, KA * P), -1, np.int16)
        idxB = np.full((W, KB * P), -1, np.int16)
        drel = np.full((W, KT * P), -1.0, np.float32)
        meta = np.zeros((1, 2 * W), np.int32)
        for w, (sa, da, sb, db) in enumerate(per_core[r]):
            na, nb = len(sa), len(sb)
            idxA[w, :na] = sa.astype(np.int16)
            idxB[w, :nb] = sb.astype(np.int16)
            if na == 0:
                idxA[w, 0] = 0
            if nb == 0:
                idxB[w, 0] = 0
            meta[0, w] = max(na, 1)
            meta[0, W + w] = max(nb, 1)
            drel[w, :na] = da.astype(np.float32)
            drel[w, KA * P:KA * P + nb] = db.astype(np.float32)

        xp = np.zeros((NLOCP, IN), np.float32)
        xp[:NLOC] = x[r * NLOC:(r + 1) * NLOC]
        in_maps.append({
            "x": xp.astype(BF16),
            "Wcat0": Wcat[0], "Wcat1": Wcat[1], "Wcat2": Wcat[2],
            "idxA": np.hstack([_wrap16(idxA[w]) for w in range(W)]),
            "idxB": np.hstack([_wrap16(idxB[w]) for w in range(W)]),
            "drel": np.hstack([drel[w].reshape(KT, P).T for w in range(W)]).astype(BF16),
            "meta": meta,
        })

    meta_prog = {"KA": KA, "KB": KB}
    return in_maps, meta_prog


# ======================= device program =======================

def _build(meta_prog):
    import concourse.bass as bass
    import concourse.bacc as bacc
    import concourse.mybir as mybir
    import concourse.tile as tile
    from concourse.masks import make_identity

    KA, KB = meta_prog["KA"], meta_prog["KB"]
    KT = KA + KB
    f32 = mybir.dt.float32
    bf16 = mybir.dt.bfloat16
    i16 = mybir.dt.int16
    i32 = mybir.dt.int32
    AF = mybir.ActivationFunctionType
    OP = mybir.AluOpType

    nc = bacc.Bacc("TRN2", target_bir_lowering=False, debug=False,
                   num_devices=NCORES)

    ROWS = (ROW0, ROW0, ROW2)
    ELS = (EL0, EL0, EL2)
    FS = (F0, F0, F2)
    RHSS = (RHS0, RHS0, RHS2)
    DCOLS = (F0 + 8, F0 + 8, F2 + 8 + F2)

    # ---- I/O ----
    x_d = nc.dram_tensor("x", [NLOCP, F0], bf16, kind="ExternalInput")
    Wc_d = [nc.dram_tensor(f"Wcat{l}", [IN, DCOLS[l]], bf16, kind="ExternalInput")
            for l in range(3)]
    idxA_d = nc.dram_tensor("idxA", [P, W * KA * 8], i16, kind="ExternalInput")
    idxB_d = nc.dram_tensor("idxB", [P, W * KB * 8], i16, kind="ExternalInput")
    drel_d = nc.dram_tensor("drel", [P, W * KT], bf16, kind="ExternalInput")
    meta_d = nc.dram_tensor("meta", [1, 2 * W], i32, kind="ExternalInput")
    out_d = nc.dram_tensor("out", [NLOC, C], f32, kind="ExternalOutput")

    # ---- internal DRAM ----
    tin = [nc.dram_tensor(f"tin{l}", [NLOC, ROWS[l]], bf16) for l in range(3)]
    tab = [nc.dram_tensor(f"tab{l}", [N, ROWS[l]], bf16, addr_space="Shared")
           for l in range(3)]
    h_d = [x_d,
           nc.dram_tensor("h1", [NLOCP, F0], bf16),
           nc.dram_tensor("h2", [NLOCP, F0], bf16)]
    res2_d = nc.dram_tensor("res2", [NLOC, F2], f32)

    with tile.TileContext(nc) as tc:
        with (
            tc.tile_pool(name="const", bufs=1) as cp,
            tc.tile_pool(name="work", bufs=2) as wp,
            tc.tile_pool(name="psum", bufs=2, space="PSUM") as pp,
        ):
            # ---- persistent tiles ----
            iota_f = cp.tile([P, P], f32)
            nc.gpsimd.iota(iota_f[:], pattern=[[1, P]], base=0,
                           channel_multiplier=0,
                           allow_small_or_imprecise_dtypes=True)
            iota_b = cp.tile([P, P], bf16)
            nc.vector.tensor_copy(out=iota_b[:], in_=iota_f[:])
            ident_f = cp.tile([P, P], f32)
            make_identity(nc, ident_f[:])
            ident_b = cp.tile([P, P], bf16)
            nc.vector.tensor_copy(out=ident_b[:], in_=ident_f[:])

            idxA_t = cp.tile([P, W * KA * 8], i16)
            nc.sync.dma_start(idxA_t[:], idxA_d[:, :])
            idxB_t = cp.tile([P, W * KB * 8], i16)
            nc.sync.dma_start(idxB_t[:], idxB_d[:, :])
            drel_t = cp.tile([P, W * KT], bf16)
            nc.sync.dma_start(drel_t[:], drel_d[:, :])
            meta_t = cp.tile([1, 2 * W], i32)
            nc.sync.dma_start(meta_t[:], meta_d[:, :])

            Wc_t = []
            for l in range(3):
                chunks = []
                for k in range(2):
                    t = cp.tile([P, DCOLS[l]], bf16, tag=f"wc{l}{k}")
                    nc.sync.dma_start(t[:], Wc_d[l][k * P:(k + 1) * P, :])
                    chunks.append(t)
                Wc_t.append(chunks)

            # gather destinations: ring of NBUF buffers; zeroed once so pad
            # lanes (not overwritten by the exact-count gathers) stay finite
            G_bufs = []
            for i in range(NBUF):
                g = cp.tile([P, KT * ROW0], bf16, tag=f"G{i}")
                nc.vector.memset(g[:, :].bitcast(f32), 0.0)
                G_bufs.append(g)

            regs = [nc.gpsimd.alloc_register(f"gr{i}") for i in range(8)]
            er_all = cp.tile([P, W, 4], f32)
            el_all = cp.tile([P, W, 4], f32)

            # zero the h1/h2 DRAM pad rows once: the dense-phase transposed
            # loads read them, and a NaN there would poison er_all via the
            # full-partition er matmul contraction
            zpad = cp.tile([NLOCP - NLOC, F0], bf16, tag="zpad")
            nc.vector.memset(zpad[:].bitcast(f32), 0.0)
            nc.sync.dma_start(h_d[1][NLOC:NLOCP, :], zpad[:])
            nc.sync.dma_start(h_d[2][NLOC:NLOCP, :], zpad[:])

            import os
            DBG = os.environ.get("GAT_DBG", "") == "1"
            DBG_L = int(os.environ.get("GAT_DBG_L", "0"))
            DBG_W = int(os.environ.get("GAT_DBG_W", "0"))

            def _tap(name, ap, shape, dtype):
                d = nc.dram_tensor(name, shape, dtype, kind="ExternalOutput")
                nc.sync.dma_start(d[tuple(slice(0, s) for s in shape)], ap)

            def dense_phase(l):
                F = FS[l]
                hview = h_d[l].rearrange("n (k p) -> n k p", p=P)
                for w in range(W):
                    nw = min(P, NLOC - w * P)
                    hT = wp.tile([P, 2, P], bf16, tag="hT")
                    for k in range(2):
                        nc.sync.dma_start(out=hT[:, k, :],
                                          in_=hview[w * P:(w + 1) * P, k, :],
                                          transpose=True)
                    psd = pp.tile([P, DCOLS[l]], f32, tag="psd")
                    for k in range(2):
                        nc.tensor.matmul(out=psd[:], lhsT=hT[:, k, :],
                                         rhs=Wc_t[l][k][:],
                                         start=(k == 0), stop=(k == 1))
                    do = wp.tile([P, ROWS[l]], bf16, tag="do")
                    nc.vector.tensor_copy(out=do[:, 0:F], in_=psd[:, 0:F])
                    nc.vector.tensor_copy(
                        out=do[:, ELS[l]:ELS[l] + 8].bitcast(f32),
                        in_=psd[:, F:F + 4])
                    nc.vector.tensor_copy(out=er_all[:, w, :],
                                          in_=psd[:, F + 4:F + 8])
                    nc.vector.tensor_copy(out=el_all[:, w, :],
                                          in_=psd[:, F:F + 4])
                    nc.scalar.dma_start(tin[l][w * P:w * P + nw, :], do[0:nw, :])
                    if DBG and l == DBG_L and w == DBG_W:
                        _tap("d_do", do[:, :], [P, ROWS[l]], bf16)
                        _tap("d_hT", hT[:, :, :], [P, 2, P], bf16)
                        psc = wp.tile([P, DCOLS[l]], f32, tag="d_psc")
                        nc.vector.tensor_copy(out=psc[:], in_=psd[:])
                        _tap("d_psd", psc[:], [P, DCOLS[l]], f32)
                    if l == 2:
                        r2 = wp.tile([P, F2], f32, tag="r2")
                        nc.vector.tensor_copy(out=r2[:], in_=psd[:, F2 + 8:F2 + 8 + F2])
                        nc.scalar.dma_start(res2_d[w * P:w * P + nw, :], r2[0:nw, :])
                    if AGSPLIT and w == H1W - 1:
                        nc.gpsimd.collective_compute(
                            "AllGather", OP.bypass,
                            replica_groups=[list(range(NCORES))],
                            ins=[tin[l][0:H1LOC, :].opt()],
                            outs=[tab[l][0:NCORES * H1LOC, :].opt()])
                    elif w == W - 1:
                        if AGSPLIT:
                            nc.gpsimd.collective_compute(
                                "AllGather", OP.bypass,
                                replica_groups=[list(range(NCORES))],
                                ins=[tin[l][H1LOC:NLOC, :].opt()],
                                outs=[tab[l][NCORES * H1LOC:N, :].opt()])
                        else:
                            nc.gpsimd.collective_compute(
                                "AllGather", OP.bypass,
                                replica_groups=[list(range(NCORES))],
                                ins=[tin[l][:, :].opt()],
                                outs=[tab[l][:, :].opt()])

            def edge_phase(l):
                ROW, EL, ER, F, RHSW = ROWS[l], ELS[l], ELS[l] + 8, FS[l], RHSS[l]
                D = F // H

                if l == 2:
                    # G stale bytes from the ROW0 layout would be reinterpreted
                    # at ROW2 stride (el slots landing mid-feat -> huge bitcast
                    # values -> exp overflow -> 0*inf NaN). Re-zero the ring.
                    for g in G_bufs:
                        nc.vector.memset(g[:, :].bitcast(f32), 0.0)

                def gath(w):
                    bufi = w % NBUF
                    G = G_bufs[bufi][:, 0:KT * ROW].rearrange(
                        "p (t c) -> p t c", c=ROW)
                    rA = regs[(2 * w) % 8]
                    rB = regs[(2 * w + 1) % 8]
                    nc.gpsimd.reg_load(rA, meta_t[0:1, w:w + 1])
                    nc.gpsimd.dma_gather(
                        out_ap=G[:, 0:KA, :], in_ap=tab[l][0:SPLIT, :],
                        idxs_ap=idxA_t[:, w * KA * 8:(w + 1) * KA * 8],
                        num_idxs=KA * P, num_idxs_reg=rA, elem_size=ROW,
                        single_packet=False)
                    nc.gpsimd.reg_load(rB, meta_t[0:1, W + w:W + w + 1])
                    nc.gpsimd.dma_gather(
                        out_ap=G[:, KA:KT, :], in_ap=tab[l][SPLIT:N, :],
                        idxs_ap=idxB_t[:, w * KB * 8:(w + 1) * KB * 8],
                        num_idxs=KB * P, num_idxs_reg=rB, elem_size=ROW,
                        single_packet=False)

                for w in range(LA):
                    gath(w)
                for w in range(W):
                    if w + LA < W:
                        gath(w + LA)
                    nw = min(P, NLOC - w * P)
                    bufi = w % NBUF
                    G = G_bufs[bufi][:, 0:KT * ROW].rearrange(
                        "p (t c) -> p t c", c=ROW)

                    oh = wp.tile([P, KT, P], bf16, tag="oh")
                    nc.vector.tensor_tensor(
                        out=oh[:, :, :],
                        in0=iota_b[:, None, :].to_broadcast([P, KT, P]),
                        in1=drel_t[:, w * KT:(w + 1) * KT, None].to_broadcast([P, KT, P]),
                        op=OP.is_equal)

                    # er broadcast to edges: per tile, transpose the one-hot
                    # and multiply by this window's per-node er (SBUF-resident)
                    erw_t = wp.tile([P, 4], bf16, tag="erwb")
                    nc.vector.tensor_copy(out=erw_t[:], in_=er_all[:, w, :])
                    er_ps = pp.tile([P, KT, 4], f32, tag="er_ps")
                    for t in range(KT):
                        oht_ps = pp.tile([P, P], bf16, tag="pst")
                        nc.tensor.transpose(out=oht_ps[:], in_=oh[:, t, :],
                                            identity=ident_b[:])
                        ohT = wp.tile([P, P], bf16, tag="ohT")
                        nc.vector.tensor_copy(out=ohT[:], in_=oht_ps[:])
                        nc.tensor.matmul(out=er_ps[:, t, :], lhsT=ohT[:],
                                         rhs=erw_t[:], start=True, stop=True)

                    ext = wp.tile([P, KT, 4], f32, tag="ext")
                    nc.vector.tensor_add(
                        ext[:, :, :],
                        G[:, :, EL:EL + 8].bitcast(f32),
                        er_ps[:, :, :])
                    nc.vector.scalar_tensor_tensor(
                        out=ext[:, :, :], in0=ext[:, :, :], scalar=SLOPE,
                        in1=ext[:, :, :], op0=OP.mult, op1=OP.max)
                    extb = wp.tile([P, KT, 4], bf16, tag="extb")
                    nc.scalar.activation(extb[:, :, :], ext[:, :, :], AF.Exp)

                    rhs = wp.tile([P, KT, RHSW], bf16, tag="rhs")
                    nc.vector.tensor_tensor(
                        out=rhs[:, :, 0:F].rearrange("p t (h d) -> p t h d", h=H),
                        in0=G[:, :, 0:F].rearrange("p t (h d) -> p t h d", h=H),
                        in1=extb[:, :, :, None].to_broadcast([P, KT, H, D]),
                        op=OP.mult)
                    nc.vector.tensor_copy(out=rhs[:, :, F:F + 4], in_=extb[:, :, :])

                    psw = pp.tile([P, RHSW], f32, tag="psw")
                    for t in range(KT):
                        nc.tensor.matmul(out=psw[:, :], lhsT=oh[:, t, :],
                                         rhs=rhs[:, t, :],
                                         start=(t == 0), stop=(t == KT - 1))

                    if DBG and l == DBG_L and w == DBG_W:
                        _tap("d_G", G[:, :, :], [P, KT, ROW], bf16)
                        erc = wp.tile([P, KT, 4], f32, tag="d_erc")
                        nc.vector.tensor_copy(out=erc[:, :, :], in_=er_ps[:, :, :])
                        _tap("d_erps", erc[:, :, :], [P, KT, 4], f32)
                        _tap("d_ext", ext[:, :, :], [P, KT, 4], f32)
                        _tap("d_extb", extb[:, :, :], [P, KT, 4], bf16)
                        _tap("d_oh", oh[:, :, :], [P, KT, P], bf16)
                        _tap("d_rhs", rhs[:, :, :], [P, KT, RHSW], bf16)
                        psb = wp.tile([P, RHSW], f32, tag="d_psb")
                        nc.vector.tensor_copy(out=psb[:], in_=psw[:, :])
                        _tap("d_psw", psb[:], [P, RHSW], f32)

                    # self-loop contribution (excluded from the gather lists):
                    # feat/el/er of the window's own nodes are all local
                    exts = wp.tile([P, 4], f32, tag="exts")
                    nc.vector.tensor_add(exts[:], el_all[:, w, :], er_all[:, w, :])
                    nc.vector.scalar_tensor_tensor(
                        out=exts[:], in0=exts[:], scalar=SLOPE,
                        in1=exts[:], op0=OP.mult, op1=OP.max)
                    nc.scalar.activation(exts[:], exts[:], AF.Exp)
                    fs = wp.tile([P, F], bf16, tag="fs")
                    nc.scalar.dma_start(fs[0:nw, :], tin[l][w * P:w * P + nw, 0:F])
                    num = wp.tile([P, F], f32, tag="num")
                    nc.vector.tensor_tensor(
                        out=num[:].rearrange("p (h d) -> p h d", h=H),
                        in0=fs[:].rearrange("p (h d) -> p h d", h=H),
                        in1=exts[:, :, None].to_broadcast([P, H, D]),
                        op=OP.mult)
                    nc.vector.tensor_add(num[:], num[:], psw[:, 0:F])

                    dn = wp.tile([P, 4], f32, tag="dn")
                    nc.vector.tensor_add(dn[:], psw[:, F:F + 4], exts[:])
                    if l < 2:
                        nc.vector.tensor_scalar_max(dn[:], dn[:], 1e-30)
                    else:
                        nc.vector.tensor_scalar(dn[:], dn[:],
                                                1e-30, float(H), OP.max, OP.mult)
                    rec = wp.tile([P, 4], f32, tag="rec")
                    nc.vector.reciprocal(rec[:], dn[:])

                    of = wp.tile([P, F], f32, tag="of")
                    nc.vector.tensor_tensor(
                        out=of[:].rearrange("p (h d) -> p h d", h=H),
                        in0=num[:].rearrange("p (h d) -> p h d", h=H),
                        in1=rec[:, :, None].to_broadcast([P, H, D]),
                        op=OP.mult)
                    if l == 1:
                        rt = wp.tile([P, F0], bf16, tag="rt")
                        nc.scalar.dma_start(rt[0:nw, :], h_d[1][w * P:w * P + nw, :])
                        nc.vector.tensor_add(of[:], of[:], rt[:])
                    elif l == 2:
                        rt2 = wp.tile([P, F2], f32, tag="rt2")
                        nc.scalar.dma_start(rt2[0:nw, :], res2_d[w * P:w * P + nw, :])
                        nc.vector.tensor_add(of[:], of[:], rt2[:])

                    if l < 2:
                        # ELU: out = (x - 1 - min(x,0)) + exp(min(x,0))
                        t0 = wp.tile([P, F0], f32, tag="t0")
                        nc.vector.tensor_scalar_min(t0[:], of[:], 0.0)
                        o1 = wp.tile([P, F0], f32, tag="o1")
                        nc.vector.scalar_tensor_tensor(
                            out=o1[:], in0=of[:], scalar=-1.0, in1=t0[:],
                            op0=OP.add, op1=OP.subtract)
                        nc.scalar.activation(t0[:], t0[:], AF.Exp)
                        hb = wp.tile([P, F0], bf16, tag="hb")
                        nc.vector.tensor_add(hb[:], o1[:], t0[:])
                        nc.scalar.dma_start(h_d[l + 1][w * P:w * P + nw, :],
                                            hb[0:nw, :])
                    else:
                        msum = wp.tile([P, C], f32, tag="msum")
                        nc.vector.tensor_reduce(
                            msum[:],
                            of[:].rearrange("p (h c) -> p c h", h=H),
                            axis=mybir.AxisListType.X, op=OP.add)
                        nc.scalar.dma_start(out_d[w * P:w * P + nw, :], msum[0:nw, :])

            for l in range(3):
                dense_phase(l)
                edge_phase(l)

            if DBG:
                d_h1 = nc.dram_tensor("d_h1", [NLOCP, F0], bf16,
                                      kind="ExternalOutput")
                nc.sync.dma_start(d_h1[:, :], h_d[1][:, :])
                d_h2 = nc.dram_tensor("d_h2", [NLOCP, F0], bf16,
                                      kind="ExternalOutput")
                nc.sync.dma_start(d_h2[:, :], h_d[2][:, :])
                d_r2 = nc.dram_tensor("d_r2", [NLOC, F2], f32,
                                      kind="ExternalOutput")
                nc.sync.dma_start(d_r2[:, :], res2_d[:, :])

    nc.compile()
    return nc


# ======================= entry point =======================

def kernel(**inputs) -> np.ndarray:
    from concourse.bass_utils import run_bass_kernel_spmd

    in_maps, meta_prog = _preprocess(inputs)
    key = (meta_prog["KA"], meta_prog["KB"])
    if key not in _CACHE:
        _CACHE[key] = _build(meta_prog)
    nc = _CACHE[key]
    res = run_bass_kernel_spmd(nc, in_maps, core_ids=list(range(NCORES)))
    return np.concatenate([r["out"] for r in res.results], axis=0)


# revision 35
# speedup vs baseline: 1.0713x; 1.0488x over previous
"""3-layer GAT on 8 Trainium2 NeuronCores (Bass/Tile) — v3.

Sharding: nodes by contiguous range (6250/core); edges by dst range. Per layer:
dense phase computes [feat|el|er] = h @ [W|W.al|W.ar] for local nodes (bf16
table rows, el kept f32 inside the row) -> AllGather the node table in two
half-shards (the first one overlaps with the second half of the dense loop) ->
edge phase gathers table[src] rows (dma_gather, int16 indices, 32768-row
table split, emitted one window ahead of the consumers), builds one-hot(dst)
tiles on DVE in bf16, broadcasts er via transposed one-hot matmuls, computes
exp(leaky_relu(el+er)) on DVE/ACT, and aggregates (weighted feature sum +
softmax denominator) with one bf16 matmul chain per 128-dst-node window into
PSUM. Epilogue normalizes, adds residual, applies ELU (or the head-mean for
the output layer).

The dense phase feeds the TensorE via HWDGE DMA-transpose loads of the bf16
activations. Softmax skips the segment-max subtraction: logits are O(1) so
exp() cannot overflow.
"""
import sys

sys.path.insert(0, "/opt/trn_rl_repo")

import os as _os
import numpy as np
import ml_dtypes

BF16 = ml_dtypes.bfloat16

# ---- problem constants (nn_GAT_3951369912452) ----
N = 50000
E = 800000
IN = 256
HID = 64
H = 4
C = 40
SLOPE = 0.2
NCORES = 8
NLOC = N // NCORES          # 6250
P = 128
W = (NLOC + P - 1) // P     # 49 windows/core
NLOCP = W * P               # 6272 (padded local rows)
H1W = 40                    # windows in AllGather half 1
H1LOC = H1W * P             # 3200
H2LOC = NLOC - H1LOC        # 3050
SPLIT = 32768               # int16 gather index limit

F0 = H * HID                # 256 feat width, layers 0/1
F2 = H * C                  # 160 feat width, layer 2
ROW0 = 384                  # bf16 table row elems, layers 0/1 (768B)
ROW2 = 256                  # layer 2 (512B)
EL0, EL2 = 256, 160         # el slot offset (bf16 elems; 4 f32 live there)
ER0, ER2 = 264, 168         # er slot offset (4 more f32)
RHS0 = 260                  # matmul rhs cols (feat + ex), layers 0/1
RHS2 = 164                  # layer 2
NBUF = 4                    # gather buffer ring
LA = 2                      # gather lookahead (windows)

_CACHE = {}
AGSPLIT = _os.environ.get("GAT_NOAGSPLIT", "") != "1"


# ======================= host preprocessing =======================

def _remap(n):
    """Global node id -> table row under the two-half AllGather layout."""
    if not AGSPLIT:
        return n
    r, ln = n // NLOC, n % NLOC
    return np.where(ln < H1LOC, r * H1LOC + ln,
                    NCORES * H1LOC + r * H2LOC + (ln - H1LOC))


def _fold_w(Wm, al, ar):
    Hh, D = al.shape
    Wal = np.stack([Wm[:, h * D:(h + 1) * D] @ al[h] for h in range(Hh)], axis=1)
    War = np.stack([Wm[:, h * D:(h + 1) * D] @ ar[h] for h in range(Hh)], axis=1)
    return Wal.astype(np.float32), War.astype(np.float32)


def _wrap16(block):
    """int16 idx list (cap,) -> dma_gather wrapped layout (128, cap//16)."""
    cap = block.shape[0]
    wb = block.reshape(cap // 16, 16).T
    return np.tile(wb, (8, 1)).astype(np.int16)


def _preprocess(inputs):
    x = np.asarray(inputs["x"], np.float32)
    src = np.asarray(inputs["src"], np.int64)
    dst = np.asarray(inputs["dst"], np.int64)

    Wcat = []
    for l, (Wm, al, ar) in enumerate(
        [(inputs["W0"], inputs["al0"], inputs["ar0"]),
         (inputs["W1"], inputs["al1"], inputs["ar1"]),
         (inputs["W2"], inputs["al2"], inputs["ar2"])]
    ):
        Wm = np.asarray(Wm, np.float32)
        Wal, War = _fold_w(Wm, np.asarray(al, np.float32), np.asarray(ar, np.float32))
        parts = [Wm, Wal, War]
        if l == 2:
            # residual projection, pre-scaled by the head-mean 1/H factor
            parts.append(np.asarray(inputs["Wres2"], np.float32) / H)
        Wcat.append(np.ascontiguousarray(np.concatenate(parts, axis=1)).astype(BF16))

    rsrc = _remap(src)
    # the appended self-loop block (one per node) is handled locally in the
    # epilogue — exclude exactly those edges from the gather lists
    selfapp = np.zeros(E, bool)
    selfapp[E - N:] = True
    order = np.argsort(dst, kind="stable")
    ds = dst[order]
    ss = rsrc[order]
    sf = selfapp[order]

    # per (core, window) edge lists
    per_core = []
    KA = KB = 1
    for r in range(NCORES):
        lo = r * NLOC
        m = (ds >= lo) & (ds < lo + NLOC) & ~sf
        ld = ds[m] - lo
        ls = ss[m]
        wins = []
        for w in range(W):
            wm = (ld >= w * P) & (ld < (w + 1) * P)
            dw = ld[wm] - w * P
            sw = ls[wm]
            a = sw < SPLIT
            sa, da = sw[a], dw[a]
            sb, db = sw[~a] - SPLIT, dw[~a]
            wins.append((sa, da, sb, db))
            KA = max(KA, -(-max(len(sa), 1) // P))
            KB = max(KB, -(-max(len(sb), 1) // P))
        per_core.append(wins)
    KT = KA + KB

    in_maps = []
    for r in range(NCORES):
        idxA = np.full((W, KA * P), -1, np.int16)
        idxB = np.full((W, KB * P), -1, np.int16)
        drel = np.full((W, KT * P), -1.0, np.float32)
        meta = np.zeros((1, 2 * W), np.int32)
        for w, (sa, da, sb, db) in enumerate(per_core[r]):
            na, nb = len(sa), len(sb)
            idxA[w, :na] = sa.astype(np.int16)
            idxB[w, :nb] = sb.astype(np.int16)
            if na == 0:
                idxA[w, 0] = 0
            if nb == 0:
                idxB[w, 0] = 0
            meta[0, w] = max(na, 1)
            meta[0, W + w] = max(nb, 1)
            drel[w, :na] = da.astype(np.float32)
            drel[w, KA * P:KA * P + nb] = db.astype(np.float32)

        xp = np.zeros((NLOCP, IN), np.float32)
        xp[:NLOC] = x[r * NLOC:(r + 1) * NLOC]
        in_maps.append({
            "x": xp.astype(BF16),
            "Wcat0": Wcat[0], "Wcat1": Wcat[1], "Wcat2": Wcat[2],
            "idxA": np.hstack([_wrap16(idxA[w]) for w in range(W)]),
            "idxB": np.hstack([_wrap16(idxB[w]) for w in range(W)]),
            "drel": np.hstack([drel[w].reshape(KT, P).T for w in range(W)]).astype(BF16),
            "meta": meta,
        })

    meta_prog = {"KA": KA, "KB": KB}
    return in_maps, meta_prog


# ======================= device program =======================

def _build(meta_prog):
    import concourse.bass as bass
    import concourse.bacc as bacc
    import concourse.mybir as mybir
    import concourse.tile as tile
    from concourse.masks import make_identity

    KA, KB = meta_prog["KA"], meta_prog["KB"]
    KT = KA + KB
    f32 = mybir.dt.float32
    bf16 = mybir.dt.bfloat16
    i16 = mybir.dt.int16
    i32 = mybir.dt.int32
    AF = mybir.ActivationFunctionType
    OP = mybir.AluOpType

    nc = bacc.Bacc("TRN2", target_bir_lowering=False, debug=False,
                   num_devices=NCORES)

    ROWS = (ROW0, ROW0, ROW2)
    ELS = (EL0, EL0, EL2)
    FS = (F0, F0, F2)
    RHSS = (RHS0, RHS0, RHS2)
    DCOLS = (F0 + 8, F0 + 8, F2 + 8 + F2)

    # ---- I/O ----
    x_d = nc.dram_tensor("x", [NLOCP, F0], bf16, kind="ExternalInput")
    Wc_d = [nc.dram_tensor(f"Wcat{l}", [IN, DCOLS[l]], bf16, kind="ExternalInput")
            for l in range(3)]
    idxA_d = nc.dram_tensor("idxA", [P, W * KA * 8], i16, kind="ExternalInput")
    idxB_d = nc.dram_tensor("idxB", [P, W * KB * 8], i16, kind="ExternalInput")
    drel_d = nc.dram_tensor("drel", [P, W * KT], bf16, kind="ExternalInput")
    meta_d = nc.dram_tensor("meta", [1, 2 * W], i32, kind="ExternalInput")
    out_d = nc.dram_tensor("out", [NLOC, C], f32, kind="ExternalOutput")

    # ---- internal DRAM ----
    tin = [nc.dram_tensor(f"tin{l}", [NLOC, ROWS[l]], bf16) for l in range(3)]
    tab = [nc.dram_tensor(f"tab{l}", [N, ROWS[l]], bf16, addr_space="Shared")
           for l in range(3)]
    h_d = [x_d,
           nc.dram_tensor("h1", [NLOCP, F0], bf16),
           nc.dram_tensor("h2", [NLOCP, F0], bf16)]
    res2_d = nc.dram_tensor("res2", [NLOC, F2], f32)

    with tile.TileContext(nc) as tc:
        with (
            tc.tile_pool(name="const", bufs=1) as cp,
            tc.tile_pool(name="work", bufs=2) as wp,
            tc.tile_pool(name="psum", bufs=2, space="PSUM") as pp,
        ):
            # ---- persistent tiles ----
            iota_f = cp.tile([P, P], f32)
            nc.gpsimd.iota(iota_f[:], pattern=[[1, P]], base=0,
                           channel_multiplier=0,
                           allow_small_or_imprecise_dtypes=True)
            iota_b = cp.tile([P, P], bf16)
            nc.vector.tensor_copy(out=iota_b[:], in_=iota_f[:])
            ident_f = cp.tile([P, P], f32)
            make_identity(nc, ident_f[:])
            ident_b = cp.tile([P, P], bf16)
            nc.vector.tensor_copy(out=ident_b[:], in_=ident_f[:])

            idxA_t = cp.tile([P, W * KA * 8], i16)
            nc.sync.dma_start(idxA_t[:], idxA_d[:, :])
            idxB_t = cp.tile([P, W * KB * 8], i16)
            nc.sync.dma_start(idxB_t[:], idxB_d[:, :])
            drel_t = cp.tile([P, W * KT], bf16)
            nc.sync.dma_start(drel_t[:], drel_d[:, :])
            meta_t = cp.tile([1, 2 * W], i32)
            nc.sync.dma_start(meta_t[:], meta_d[:, :])

            Wc_t = []
            for l in range(3):
                chunks = []
                for k in range(2):
                    t = cp.tile([P, DCOLS[l]], bf16, tag=f"wc{l}{k}")
                    nc.sync.dma_start(t[:], Wc_d[l][k * P:(k + 1) * P, :])
                    chunks.append(t)
                Wc_t.append(chunks)

            # gather destinations: ring of NBUF buffers; zeroed once so pad
            # lanes (not overwritten by the exact-count gathers) stay finite
            G_bufs = []
            for i in range(NBUF):
                g = cp.tile([P, KT * ROW0], bf16, tag=f"G{i}")
                nc.vector.memset(g[:, :].bitcast(f32), 0.0)
                G_bufs.append(g)

            regs = [nc.gpsimd.alloc_register(f"gr{i}") for i in range(8)]
            er_all = cp.tile([P, W, 4], f32)
            el_all = cp.tile([P, W, 4], f32)

            # zero the h1/h2 DRAM pad rows once: the dense-phase transposed
            # loads read them, and a NaN there would poison er_all via the
            # full-partition er matmul contraction
            zpad = cp.tile([NLOCP - NLOC, F0], bf16, tag="zpad")
            nc.vector.memset(zpad[:].bitcast(f32), 0.0)
            nc.sync.dma_start(h_d[1][NLOC:NLOCP, :], zpad[:])
            nc.sync.dma_start(h_d[2][NLOC:NLOCP, :], zpad[:])

            import os
            DBG = os.environ.get("GAT_DBG", "") == "1"
            DBG_L = int(os.environ.get("GAT_DBG_L", "0"))
            DBG_W = int(os.environ.get("GAT_DBG_W", "0"))

            def _tap(name, ap, shape, dtype):
                d = nc.dram_tensor(name, shape, dtype, kind="ExternalOutput")
                nc.sync.dma_start(d[tuple(slice(0, s) for s in shape)], ap)

            def dense_phase(l):
                F = FS[l]
                hview = h_d[l].rearrange("n (k p) -> n k p", p=P)
                for w in range(W):
                    nw = min(P, NLOC - w * P)
                    hT = wp.tile([P, 2, P], bf16, tag="hT")
                    for k in range(2):
                        nc.sync.dma_start(out=hT[:, k, :],
                                          in_=hview[w * P:(w + 1) * P, k, :],
                                          transpose=True)
                    psd = pp.tile([P, DCOLS[l]], f32, tag="psd")
                    for k in range(2):
                        nc.tensor.matmul(out=psd[:], lhsT=hT[:, k, :],
                                         rhs=Wc_t[l][k][:],
                                         start=(k == 0), stop=(k == 1))
                    do = wp.tile([P, ROWS[l]], bf16, tag="do")
                    nc.vector.tensor_copy(out=do[:, 0:F], in_=psd[:, 0:F])
                    nc.vector.tensor_copy(
                        out=do[:, ELS[l]:ELS[l] + 8].bitcast(f32),
                        in_=psd[:, F:F + 4])
                    nc.vector.tensor_copy(out=er_all[:, w, :],
                                          in_=psd[:, F + 4:F + 8])
                    nc.vector.tensor_copy(out=el_all[:, w, :],
                                          in_=psd[:, F:F + 4])
                    nc.scalar.dma_start(tin[l][w * P:w * P + nw, :], do[0:nw, :])
                    if DBG and l == DBG_L and w == DBG_W:
                        _tap("d_do", do[:, :], [P, ROWS[l]], bf16)
                        _tap("d_hT", hT[:, :, :], [P, 2, P], bf16)
                        psc = wp.tile([P, DCOLS[l]], f32, tag="d_psc")
                        nc.vector.tensor_copy(out=psc[:], in_=psd[:])
                        _tap("d_psd", psc[:], [P, DCOLS[l]], f32)
                    if l == 2:
                        r2 = wp.tile([P, F2], f32, tag="r2")
                        nc.vector.tensor_copy(out=r2[:], in_=psd[:, F2 + 8:F2 + 8 + F2])
                        nc.scalar.dma_start(res2_d[w * P:w * P + nw, :], r2[0:nw, :])
                    if AGSPLIT and w == H1W - 1:
                        nc.gpsimd.collective_compute(
                            "AllGather", OP.bypass,
                            replica_groups=[list(range(NCORES))],
                            ins=[tin[l][0:H1LOC, :].opt()],
                            outs=[tab[l][0:NCORES * H1LOC, :].opt()])
                    elif w == W - 1:
                        if AGSPLIT:
                            nc.gpsimd.collective_compute(
                                "AllGather", OP.bypass,
                                replica_groups=[list(range(NCORES))],
                                ins=[tin[l][H1LOC:NLOC, :].opt()],
                                outs=[tab[l][NCORES * H1LOC:N, :].opt()])
                        else:
                            nc.gpsimd.collective_compute(
                                "AllGather", OP.bypass,
                                replica_groups=[list(range(NCORES))],
                                ins=[tin[l][:, :].opt()],
                                outs=[tab[l][:, :].opt()])

            def edge_phase(l):
                ROW, EL, ER, F, RHSW = ROWS[l], ELS[l], ELS[l] + 8, FS[l], RHSS[l]
                D = F // H

                if l == 2:
                    # G stale bytes from the ROW0 layout would be reinterpreted
                    # at ROW2 stride (el slots landing mid-feat -> huge bitcast
                    # values -> exp overflow -> 0*inf NaN). Re-zero the ring.
                    for g in G_bufs:
                        nc.vector.memset(g[:, :].bitcast(f32), 0.0)

                def gath(w):
                    bufi = w % NBUF
                    G = G_bufs[bufi][:, 0:KT * ROW].rearrange(
                        "p (t c) -> p t c", c=ROW)
                    rA = regs[(2 * w) % 8]
                    rB = regs[(2 * w + 1) % 8]
                    nc.gpsimd.reg_load(rA, meta_t[0:1, w:w + 1])
                    nc.gpsimd.dma_gather(
                        out_ap=G[:, 0:KA, :], in_ap=tab[l][0:SPLIT, :],
                        idxs_ap=idxA_t[:, w * KA * 8:(w + 1) * KA * 8],
                        num_idxs=KA * P, num_idxs_reg=rA, elem_size=ROW,
                        single_packet=False)
                    nc.gpsimd.reg_load(rB, meta_t[0:1, W + w:W + w + 1])
                    nc.gpsimd.dma_gather(
                        out_ap=G[:, KA:KT, :], in_ap=tab[l][SPLIT:N, :],
                        idxs_ap=idxB_t[:, w * KB * 8:(w + 1) * KB * 8],
                        num_idxs=KB * P, num_idxs_reg=rB, elem_size=ROW,
                        single_packet=False)

                for w in range(LA):
                    gath(w)
                for w in range(W):
                    if w + LA < W:
                        gath(w + LA)
                    nw = min(P, NLOC - w * P)
                    bufi = w % NBUF
                    G = G_bufs[bufi][:, 0:KT * ROW].rearrange(
                        "p (t c) -> p t c", c=ROW)

                    oh = wp.tile([P, KT, P], bf16, tag="oh")
                    nc.vector.tensor_tensor(
                        out=oh[:, :, :],
                        in0=iota_b[:, None, :].to_broadcast([P, KT, P]),
                        in1=drel_t[:, w * KT:(w + 1) * KT, None].to_broadcast([P, KT, P]),
                        op=OP.is_equal)

                    # er broadcast to edges: per tile, transpose the one-hot
                    # and multiply by this window's per-node er (SBUF-resident)
                    erw_t = wp.tile([P, 4], bf16, tag="erwb")
                    nc.vector.tensor_copy(out=erw_t[:], in_=er_all[:, w, :])
                    er_ps = pp.tile([P, KT, 4], f32, tag="er_ps")
                    for t in range(KT):
                        oht_ps = pp.tile([P, P], bf16, tag="pst")
                        nc.tensor.transpose(out=oht_ps[:], in_=oh[:, t, :],
                                            identity=ident_b[:])
                        ohT = wp.tile([P, P], bf16, tag="ohT")
                        nc.vector.tensor_copy(out=ohT[:], in_=oht_ps[:])
                        nc.tensor.matmul(out=er_ps[:, t, :], lhsT=ohT[:],
                                         rhs=erw_t[:], start=True, stop=True)

                    ext = wp.tile([P, KT, 4], f32, tag="ext")
                    nc.vector.tensor_add(
                        ext[:, :, :],
                        G[:, :, EL:EL + 8].bitcast(f32),
                        er_ps[:, :, :])
                    nc.vector.scalar_tensor_tensor(
                        out=ext[:, :, :], in0=ext[:, :, :], scalar=SLOPE,
                        in1=ext[:, :, :], op0=OP.mult, op1=OP.max)
                    extb = wp.tile([P, KT, 4], bf16, tag="extb")
                    nc.scalar.activation(extb[:, :, :], ext[:, :, :], AF.Exp)

                    rhs = wp.tile([P, KT, RHSW], bf16, tag="rhs")
                    nc.vector.tensor_tensor(
                        out=rhs[:, :, 0:F].rearrange("p t (h d) -> p t h d", h=H),
                        in0=G[:, :, 0:F].rearrange("p t (h d) -> p t h d", h=H),
                        in1=extb[:, :, :, None].to_broadcast([P, KT, H, D]),
                        op=OP.mult)
                    nc.vector.tensor_copy(out=rhs[:, :, F:F + 4], in_=extb[:, :, :])

                    psw = pp.tile([P, RHSW], f32, tag="psw")
                    for t in range(KT):
                        nc.tensor.matmul(out=psw[:, :], lhsT=oh[:, t, :],
                                         rhs=rhs[:, t, :],
                                         start=(t == 0), stop=(t == KT - 1))

                    if DBG and l == DBG_L and w == DBG_W:
                        _tap("d_G", G[:, :, :], [P, KT, ROW], bf16)
                        erc = wp.tile([P, KT, 4], f32, tag="d_erc")
                        nc.vector.tensor_copy(out=erc[:, :, :], in_=er_ps[:, :, :])
                        _tap("d_erps", erc[:, :, :], [P, KT, 4], f32)
                        _tap("d_ext", ext[:, :, :], [P, KT, 4], f32)
                        _tap("d_extb", extb[:, :, :], [P, KT, 4], bf16)
                        _tap("d_oh", oh[:, :, :], [P, KT, P], bf16)
                        _tap("d_rhs", rhs[:, :, :], [P, KT, RHSW], bf16)
                        psb = wp.tile([P, RHSW], f32, tag="d_psb")
                        nc.vector.tensor_copy(out=psb[:], in_=psw[:, :])
                        _tap("d_psw", psb[:], [P, RHSW], f32)

                    # self-loop contribution (excluded from the gather lists):
                    # feat/el/er of the window's own nodes are all local
                    exts = wp.tile([P, 4], f32, tag="exts")
                    nc.vector.tensor_add(exts[:], el_all[:, w, :], er_all[:, w, :])
                    nc.vector.scalar_tensor_tensor(
                        out=exts[:], in0=exts[:], scalar=SLOPE,
                        in1=exts[:], op0=OP.mult, op1=OP.max)
                    nc.scalar.activation(exts[:], exts[:], AF.Exp)
                    fs = wp.tile([P, F], bf16, tag="fs")
                    nc.scalar.dma_start(fs[0:nw, :], tin[l][w * P:w * P + nw, 0:F])
                    num = wp.tile([P, F], f32, tag="num")
                    nc.vector.tensor_tensor(
                        out=num[:].rearrange("p (h d) -> p h d", h=H),
                        in0=fs[:].rearrange("p (h d) -> p h d", h=H),
                        in1=exts[:, :, None].to_broadcast([P, H, D]),
                        op=OP.mult)
                    nc.vector.tensor_add(num[:], num[:], psw[:, 0:F])

                    dn = wp.tile([P, 4], f32, tag="dn")
                    nc.vector.tensor_add(dn[:], psw[:, F:F + 4], exts[:])
                    if l < 2:
                        nc.vector.tensor_scalar_max(dn[:], dn[:], 1e-30)
                    else:
                        nc.vector.tensor_scalar(dn[:], dn[:],
                                                1e-30, float(H), OP.max, OP.mult)
                    rec = wp.tile([P, 4], f32, tag="rec")
                    nc.vector.reciprocal(rec[:], dn[:])

                    of = wp.tile([P, F], f32, tag="of")
                    nc.vector.tensor_tensor(
                        out=of[:].rearrange("p (h d) -> p h d", h=H),
                        in0=num[:].rearrange("p (h d) -> p h d", h=H),
                        in1=rec[:, :, None].to_broadcast([P, H, D]),
                        op=OP.mult)
                    if l == 1:
                        rt = wp.tile([P, F0], bf16, tag="rt")
                        nc.scalar.dma_start(rt[0:nw, :], h_d[1][w * P:w * P + nw, :])
                        nc.vector.tensor_add(of[:], of[:], rt[:])
                    elif l == 2:
                        rt2 = wp.tile([P, F2], f32, tag="rt2")
                        nc.scalar.dma_start(rt2[0:nw, :], res2_d[w * P:w * P + nw, :])
                        nc.vector.tensor_add(of[:], of[:], rt2[:])

                    if l < 2:
                        # ELU: out = (x - 1 - min(x,0)) + exp(min(x,0))
                        t0 = wp.tile([P, F0], f32, tag="t0")
                        nc.vector.tensor_scalar_min(t0[:], of[:], 0.0)
                        o1 = wp.tile([P, F0], f32, tag="o1")
                        nc.vector.scalar_tensor_tensor(
                            out=o1[:], in0=of[:], scalar=-1.0, in1=t0[:],
                            op0=OP.add, op1=OP.subtract)
                        nc.scalar.activation(t0[:], t0[:], AF.Exp)
                        hb = wp.tile([P, F0], bf16, tag="hb")
                        nc.vector.tensor_add(hb[:], o1[:], t0[:])
                        nc.scalar.dma_start(h_d[l + 1][w * P:w * P + nw, :],
                                            hb[0:nw, :])
                    else:
                        msum = wp.tile([P, C], f32, tag="msum")
                        nc.vector.tensor_reduce(
                            msum[:],
                            of[:].rearrange("p (h c) -> p c h", h=H),
                            axis=mybir.AxisListType.X, op=OP.add)
                        nc.scalar.dma_start(out_d[w * P:w * P + nw, :], msum[0:nw, :])

            for l in range(3):
                dense_phase(l)
                edge_phase(l)

            if DBG:
                d_h1 = nc.dram_tensor("d_h1", [NLOCP, F0], bf16,
                                      kind="ExternalOutput")
                nc.sync.dma_start(d_h1[:, :], h_d[1][:, :])
                d_h2 = nc.dram_tensor("d_h2", [NLOCP, F0], bf16,
                                      kind="ExternalOutput")
                nc.sync.dma_start(d_h2[:, :], h_d[2][:, :])
                d_r2 = nc.dram_tensor("d_r2", [NLOC, F2], f32,
                                      kind="ExternalOutput")
                nc.sync.dma_start(d_r2[:, :], res2_d[:, :])

    nc.compile()
    return nc


# ======================= entry point =======================

def kernel(**inputs) -> np.ndarray:
    from concourse.bass_utils import run_bass_kernel_spmd

    in_maps, meta_prog = _preprocess(inputs)
    key = (meta_prog["KA"], meta_prog["KB"])
    if key not in _CACHE:
        _CACHE[key] = _build(meta_prog)
    nc = _CACHE[key]
    res = run_bass_kernel_spmd(nc, in_maps, core_ids=list(range(NCORES)))
    return np.concatenate([r["out"] for r in res.results], axis=0)
